# revision 9
# baseline (speedup 1.0000x reference)
"""Causal self-attention (GPT-2 block) for Trainium2, 8 NeuronCores.

Sharding: core = 2*batch + head_group. Each of the 8 cores handles one of
B=4 batches and one group of 8 of the 16 heads (Megatron column-split of
the QKV weights, row-split of the proj weights). The two head-group
partial proj outputs per batch are summed on the host; the V-bias and
proj-bias terms are folded into a single host-side additive correction
(softmax rows sum to 1, so attn @ (1 x bv) == bv broadcast).

All matmul operands are bf16 (f32r streams the moving operand at ~2
cycles/column on TRN2 silicon; bf16 streams at 1 and enables FWL weight
loads). PSUM accumulation stays fp32.

On-core layout:
  xt    [128, 8, 256] per chunk (all 8 kept)  x^T via PE transpose of the
                      DVE-cast bf16 x chunk
  QT/KT [128, 4, S]   feature-major: partition p, slice j <-> feature
                      j*128+p; head h at partitions (h%2)*64.., slice h//2
  V     [128, 16, 8, 65]  natural [s, feat] per head + ones column
  attnT [128, 4, SC]  attention output, feature-major (proj stationary)

Schedule (the key difference from the chunk-ordered variant): Q for ALL
of S is computed first (phase A: x load/cast/transpose + Q per 256-chunk).
Phase B computes K and V per 256-chunk; after KV chunk 2q+1, the full
per-(head-pair, q-chunk) attention sweeps for q-chunk q are enqueued and
dripped between the K/V matmul groups of later chunks. This removes the
segregated attention tail (which measured HAM-cold at 1.2 GHz) — the tail
is only the q=3 sweeps + proj, all dense back-to-back PE work.

Attention per (pair, q-chunk) sweep: per k-block, both heads' score
matmuls [128 k, <=512 q] land in one [128, 2, 512] fp32 PSUM tile
(2 banks) so a single ScalarE exp call covers both heads (halves the
per-call ACTIVATE overhead); additive -1e30 causal mask on diagonal
corners; PV + row sums via the V ones-column accumulate into per-head
[65, 512] PSUM; then a parallel-lane reciprocal (DMA-scattered to
[128,4]) and a DRAM-bounced partition broadcast normalize.
"""

import os

import numpy as np

import concourse.bass as bass
import concourse.tile as tile
from concourse import bacc, mybir
from concourse.bass_utils import run_bass_kernel_spmd
from concourse.masks import make_identity, make_lower_triangular

# Problem shape (fixed by the harness contract).
B, S, D, H, HD = 4, 2048, 1024, 16, 64
NCORES = 8
HG = 8                # heads per core
FG = HG * HD          # 512 features per head group
P = 128
DB = D // P           # 8 contraction blocks
FBN = FG // P         # 4 feature blocks
SC = 512              # attention sequence chunk
NQ = S // SC          # 4
NKB = S // P          # 16 key blocks
XC = 256              # QKV s-chunk width
NXC = S // XC         # 8
F32 = mybir.dt.float32
BF16 = mybir.dt.bfloat16
DT_MM = BF16
EXP = mybir.ActivationFunctionType.Exp
SCALE = 1.0 / float(HD) ** 0.5
MASKVAL = -1e30


class _Ctx:
    """Tiles/pools shared by the emission thunks."""


def _band_thunks(nc, cx, pair, qq, kb_lo, kb_hi, first, last):
    """Thunks for one band (k-blocks kb_lo..kb_hi-1) of the attention
    sweep for one head pair and q-chunk. Bands accumulate PV+rowsums in
    PSUM; non-final bands park partials in bf16 SBUF tiles which the
    final band merges before the normalize. Blocks are split into
    scores / exp / PV thunks so dripped K/V matmuls can fill the PE
    bubbles of the serial exp chain."""
    hA, hB = 2 * pair, 2 * pair + 1
    heads = ((hA, 0, pair), (hB, 64, pair))   # (head, pb, slice j)
    kbs = list(range(kb_lo, kb_hi))
    nblk = len(kbs)
    st = {}
    thunks = []

    def setup():
        st["ps"] = [cx.psout.tile([65, SC], F32, tag="outps",
                                  name=f"outps{sl}") for sl in range(2)]
    thunks.append(setup)

    for i, kb in enumerate(kbs):
        def scores(i=i, kb=kb):
            jj = kb - 4 * qq          # 0..3 => diagonal block
            off = jj * P if jj >= 0 else 0
            psc = cx.pscp.tile([P, 2, SC], F32, tag="psc")
            st["psc"] = psc
            for sl, (h, pb, j) in enumerate(heads):
                nc.tensor.matmul(
                    psc[:, sl, off:],
                    cx.KT[pb:pb + 64, j, kb * P:(kb + 1) * P],
                    cx.QTc[qq][pb:pb + 64, j, off:SC],
                    start=True, stop=True, tile_position=(pb, 0))
            if jj >= 0:
                for sl in range(2):
                    nc.vector.tensor_add(
                        psc[:, sl, off:off + P],
                        psc[:, sl, off:off + P], cx.addmask)

        def expblk(i=i, kb=kb):
            jj = kb - 4 * qq
            off = jj * P if jj >= 0 else 0
            psc = st["psc"]
            sx = cx.sxp.tile([P, 2, SC], DT_MM, tag="sx")
            st["sx"] = sx
            if off == 0:
                # One ScalarE call covers both heads' scores (2 banks).
                nc.scalar.activation(sx, psc, EXP, scale=SCALE)
            else:
                for sl in range(2):
                    nc.scalar.activation(
                        sx[:, sl, off:], psc[:, sl, off:], EXP, scale=SCALE)

        def pv(i=i, kb=kb):
            jj = kb - 4 * qq
            off = jj * P if jj >= 0 else 0
            sx = st["sx"]
            for sl, (h, pb, j) in enumerate(heads):
                nc.tensor.matmul(
                    st["ps"][sl][:, off:], cx.V[:, kb, h, :],
                    sx[:, sl, off:],
                    start=(i == 0), stop=(i == nblk - 1))
        thunks += [scores, expblk, pv]

    if not last:
        def pdrain():
            for sl, (h, pb, j) in enumerate(heads):
                part = cx.partp.tile([65, SC], F32, tag="part",
                                     name=f"part{pair}_{qq}_{sl}")
                if first:
                    nc.vector.tensor_copy(part, st["ps"][sl])
                else:
                    nc.vector.tensor_add(
                        part, cx.parts[(pair, qq, sl)], st["ps"][sl])
                cx.parts[(pair, qq, sl)] = part
        thunks.append(pdrain)
        return thunks

    def drain():
        st["raws"] = []
        for sl in range(2):
            raw = cx.nrmraw.tile([65, SC], F32, tag="raw")
            if first:
                nc.vector.tensor_copy(raw, st["ps"][sl])
            else:
                nc.vector.tensor_add(
                    raw, cx.parts[(pair, qq, sl)], st["ps"][sl])
            st["raws"].append(raw)

    def norm():
        for (h, pb, j), raw in zip(heads, st["raws"]):
            # Single-partition reciprocal blocks the DVE FIFO for ~us;
            # DMA-scatter the sums across 128 partitions first.
            rsh = cx.nrmbc.tile([P, SC // P], F32, tag="rsh")
            nc.sync.dma_start(rsh, raw[64:65, :])
            nc.vector.reciprocal(rsh, rsh)
            rdram = cx.drp.tile([1, SC], F32, tag="rdram")
            nc.sync.dma_start(rdram, rsh)
            rb = cx.nrmbc.tile([64, SC], F32, tag="rb")
            nc.sync.dma_start(rb, rdram.to_broadcast([64, SC]))
            stg = cx.nrmbc.tile([64, SC], DT_MM, tag="stg")
            nc.vector.tensor_mul(stg, raw[0:64, :], rb)
            nc.sync.dma_start(cx.attnTc[qq][pb:pb + 64, j, :], stg)

    thunks += [drain, norm]
    return thunks


def _proj_chunk_thunks(nc, cx, q, out_d):
    """Proj for the s-blocks of chunk q; two thunks per s-block."""
    thunks = []
    for sb in range(SC // P):
        sblk = q * (SC // P) + sb

        def make_half(hf, sblk=sblk, sb=sb):
            def run():
                og = cx.ogp.tile([P, D // 2], F32, tag="og")
                ps = cx.ps1.tile([P, D // 2], F32, tag="qkps")
                n0 = hf * (D // 2)
                for j in range(FBN):
                    nc.tensor.matmul(
                        ps,
                        cx.attnTc[q][:, j, sb * P:(sb + 1) * P],
                        cx.wp_sb[:, j, n0:n0 + D // 2],
                        start=(j == 0), stop=(j == FBN - 1))
                nc.any.tensor_copy(og, ps)
                nc.sync.dma_start(
                    out_d.ap()[sblk * P:(sblk + 1) * P, n0:n0 + D // 2], og)
            return run

        thunks.append(make_half(0))
        thunks.append(make_half(1))
    return thunks


def _body(tc, x_d, wq_d, wk_d, wv_d, wp_d, bq_d, bk_d, out_d):
    nc = tc.nc
    cx = _Ctx()
    with (
        tc.tile_pool(name="persist", bufs=1) as persist,
        tc.tile_pool(name="ph1", bufs=1) as ph1,
        tc.tile_pool(name="xin", bufs=3) as xinp,
        tc.tile_pool(name="xbp", bufs=3) as xbp,
        tc.tile_pool(name="xtp", bufs=NXC) as xtp,
        tc.tile_pool(name="qtc", bufs=NQ) as qtc,
        tc.tile_pool(name="atc", bufs=2) as atc,
        tc.tile_pool(name="sxp", bufs=3) as sxp,
        tc.tile_pool(name="nrmraw", bufs=3) as nrmraw,
        tc.tile_pool(name="nrmbc", bufs=2) as nrmbc,
        tc.tile_pool(name="partp", bufs=16) as partp,
        tc.tile_pool(name="ogp", bufs=2) as ogp,
        # PSUM banks: qkps(+transpose) 2 + psc 2x2 + outps 2 = 8
        tc.tile_pool(name="ps1", bufs=2, space="PSUM") as ps1,
        tc.tile_pool(name="pscp", bufs=2, space="PSUM") as pscp,
        tc.tile_pool(name="psout", bufs=2, space="PSUM") as psout,
        tc.tile_pool(name="drp", bufs=8, space="DRAM") as drp,
    ):
        cx.sxp, cx.nrmraw, cx.nrmbc, cx.ogp = sxp, nrmraw, nrmbc, ogp
        cx.pscp, cx.psout, cx.drp, cx.ps1 = pscp, psout, drp, ps1
        cx.partp, cx.parts = partp, {}

        ident = persist.tile([P, P], DT_MM)
        make_identity(nc, ident)
        for _ in range(12):
            wp_ps = ps1.tile([P, P], F32, tag="qkps")
            nc.tensor.matmul(wp_ps, ident, ident, start=True, stop=True)
        cx.addmask = persist.tile([P, P], F32)
        make_lower_triangular(nc, cx.addmask, val=MASKVAL, diag=False)
        bq_sb = persist.tile([P, FBN], F32)
        bk_sb = persist.tile([P, FBN], F32)
        nc.sync.dma_start(bq_sb, bq_d.ap().rearrange("(j p) -> p j", p=P))
        nc.sync.dma_start(bk_sb, bk_d.ap().rearrange("(j p) -> p j", p=P))

        cx.KT = persist.tile([P, FBN, S], DT_MM)
        cx.V = persist.tile([P, NKB, HG, HD + 1], DT_MM)
        ones_col = persist.tile([P, 1], F32)
        nc.vector.memset(ones_col, 1.0)
        nc.vector.tensor_copy(cx.V[:, :, :, HD],
                              ones_col.to_broadcast([P, NKB, HG]))
        cx.wp_sb = persist.tile([P, FBN, D], DT_MM)
        cx.QTc = [qtc.tile([P, FBN, SC], DT_MM, tag="qtc", name=f"qtc{q}")
                  for q in range(NQ)]
        cx.attnTc = [atc.tile([P, FBN, SC], DT_MM, tag="atc",
                              name=f"atc{q}") for q in range(NQ)]

        wq_sb = ph1.tile([P, DB, FG], DT_MM)
        wk_sb = ph1.tile([P, DB, FG], DT_MM)
        wv_sb = ph1.tile([P, DB, FG], DT_MM)
        xts = [xtp.tile([P, DB, XC], DT_MM, tag="xt", name=f"xt{c}")
               for c in range(NXC)]

        bg = []

        def drip(n):
            for _ in range(n):
                if bg:
                    bg.pop(0)()

        state = {"per": 1}

        def t_chunk(c):
            """x DMA + bf16 cast + PE transposes for one 256-chunk."""
            for sb in range(XC // P):
                s0 = c * XC + sb * P
                for dh in range(2):
                    xin = xinp.tile([P, D // 2], F32, tag="xin")
                    nc.sync.dma_start(
                        xin, x_d.ap()[s0:s0 + P,
                                      dh * (D // 2):(dh + 1) * (D // 2)])
                    xb = xbp.tile([P, D // 2], DT_MM, tag="xb")
                    nc.vector.tensor_copy(xb, xin)
                    if c == 0:
                        # Paced pre-warm: junk matmuls keyed to the input
                        # DMAs keep the PE HAM busy through the load window.
                        wp_ps = ps1.tile([P, P], F32, tag="qkps")
                        nc.tensor.matmul(wp_ps, ident, xb[:, 0:P],
                                         start=True, stop=True)
                    for db4 in range(DB // 2):
                        db = dh * (DB // 2) + db4
                        pt = ps1.tile([P, P], DT_MM, tag="qkps")
                        nc.tensor.transpose(
                            pt, xb[:, db4 * P:(db4 + 1) * P], ident)
                        nc.any.tensor_copy(
                            xts[c][:, db, sb * P:(sb + 1) * P], pt)

        def q_chunk(c):
            qq, half = divmod(c, 2)
            for fb in range(FBN):
                ps = ps1.tile([P, XC], F32, tag="qkps")
                for db in range(DB):
                    nc.tensor.matmul(
                        ps,
                        wq_sb[:, db, fb * P:(fb + 1) * P],
                        xts[c][:, db, :],
                        start=(db == 0), stop=(db == DB - 1))
                    drip(state["per"] if db % 2 else 0)
                nc.any.tensor_scalar_add(
                    cx.QTc[qq][:, fb, half * XC:(half + 1) * XC], ps,
                    bq_sb[:, fb:fb + 1])
                drip(state["per"])

        def k_chunk(c):
            for fb in range(FBN):
                ps = ps1.tile([P, XC], F32, tag="qkps")
                for db in range(DB):
                    nc.tensor.matmul(
                        ps,
                        wk_sb[:, db, fb * P:(fb + 1) * P],
                        xts[c][:, db, :],
                        start=(db == 0), stop=(db == DB - 1))
                    drip(state["per"] if db % 2 else 0)
                nc.vector.tensor_scalar_add(
                    cx.KT[:, fb, c * XC:(c + 1) * XC], ps,
                    bk_sb[:, fb:fb + 1])
                drip(state["per"])

        def v_chunk(c):
            for sb in range(XC // P):
                kb = c * (XC // P) + sb
                ps = ps1.tile([P, FG], F32, tag="qkps")
                for db in range(DB):
                    nc.tensor.matmul(
                        ps,
                        xts[c][:, db, sb * P:(sb + 1) * P],
                        wv_sb[:, db, :],
                        start=(db == 0), stop=(db == DB - 1))
                    drip(state["per"] if db % 2 else 0)
                nc.vector.tensor_copy(
                    cx.V[:, kb, :, 0:HD],
                    ps.rearrange("p (h c) -> p h c", h=HG))
                drip(state["per"])

        def bands(qq, kb_lo, kb_hi, first, last):
            for pr in range(HG // 2):
                bg.extend(_band_thunks(nc, cx, pr, qq, kb_lo, kb_hi,
                                       first, last))

        # x for the first chunks is in flight before the weights so the
        # transposes (which need no weights) can start immediately.
        t_chunk(0)
        t_chunk(1)
        for w_sb, w_d in ((wq_sb, wq_d), (wk_sb, wk_d), (wv_sb, wv_d)):
            wr = w_d.ap().rearrange("(db p) f -> db p f", p=P)
            for db in range(DB):
                nc.sync.dma_start(w_sb[:, db], wr[db])
        nc.sync.dma_start(
            cx.wp_sb, wp_d.ap().rearrange("(j p) n -> p j n", p=P))

        # Front: transposes + Q run 4 chunks ahead of K/V.
        q_chunk(0)
        q_chunk(1)
        t_chunk(2)
        q_chunk(2)
        t_chunk(3)
        q_chunk(3)

        # Steady: K/V per chunk; banded attention sweeps enqueued at
        # earliest readiness and dripped between the matmul groups.
        for c in range(NXC):
            state["per"] = max(1, (len(bg) + 54) // 55)
            k_chunk(c)
            v_chunk(c)
            if c + 4 < NXC:
                t_chunk(c + 4)
                q_chunk(c + 4)
            if c == 1:
                bands(0, 0, 4, True, True)
                bg.extend(_proj_chunk_thunks(nc, cx, 0, out_d))
            elif c == 3:
                bands(1, 0, 8, True, True)
                bg.extend(_proj_chunk_thunks(nc, cx, 1, out_d))
                bands(2, 0, 8, True, False)
                bands(3, 0, 8, True, False)
            elif c == 5:
                bands(2, 8, 12, False, True)
                bg.extend(_proj_chunk_thunks(nc, cx, 2, out_d))
            elif c == 7:
                bands(3, 8, 16, False, True)
                bg.extend(_proj_chunk_thunks(nc, cx, 3, out_d))

        # Tail: final q=3 band + proj, dense back-to-back.
        while bg:
            bg.pop(0)()


def build_nc():
    nc = bacc.Bacc("TRN2", target_bir_lowering=False)
    x_d = nc.dram_tensor("x", [S, D], F32, kind="ExternalInput")
    wq_d = nc.dram_tensor("wq", [D, FG], DT_MM, kind="ExternalInput")
    wk_d = nc.dram_tensor("wk", [D, FG], DT_MM, kind="ExternalInput")
    wv_d = nc.dram_tensor("wv", [D, FG], DT_MM, kind="ExternalInput")
    wp_d = nc.dram_tensor("wp", [FG, D], DT_MM, kind="ExternalInput")
    bq_d = nc.dram_tensor("bq", [FG], F32, kind="ExternalInput")
    bk_d = nc.dram_tensor("bk", [FG], F32, kind="ExternalInput")
    out_d = nc.dram_tensor("out", [S, D], F32, kind="ExternalOutput")
    with tile.TileContext(nc) as tc:
        _body(tc, x_d, wq_d, wk_d, wv_d, wp_d, bq_d, bk_d, out_d)
    nc.compile()
    return nc


_NC = None


def _get_nc():
    global _NC
    if _NC is None:
        _NC = build_nc()
    return _NC


def make_in_maps(hs, w, bvec, pw):
    import ml_dtypes
    wdt = ml_dtypes.bfloat16 if DT_MM == BF16 else np.float32
    in_maps = []
    for core in range(NCORES):
        b, g = divmod(core, 2)
        lo, hi = g * FG, (g + 1) * FG
        in_maps.append({
            "x": np.ascontiguousarray(hs[b]),
            "wq": np.ascontiguousarray(w[:, lo:hi]).astype(wdt),
            "wk": np.ascontiguousarray(w[:, D + lo:D + hi]).astype(wdt),
            "wv": np.ascontiguousarray(
                w[:, 2 * D + lo:2 * D + hi]).astype(wdt),
            "wp": np.ascontiguousarray(pw[lo:hi, :]).astype(wdt),
            "bq": np.ascontiguousarray(bvec[lo:hi]),
            "bk": np.ascontiguousarray(bvec[D + lo:D + hi]),
        })
    return in_maps


def combine(parts, bvec, pw, pb):
    bv = bvec[2 * D:3 * D].astype(np.float64)
    corr = (bv @ pw.astype(np.float64) + pb.astype(np.float64)).astype(
        np.float32)
    out = np.empty((B, S, D), np.float32)
    for b in range(B):
        out[b] = parts[2 * b] + parts[2 * b + 1] + corr
    return out


def kernel(hidden_states, c_attn_w, c_attn_b, c_proj_w, c_proj_b,
           **run_kwargs):
    hs = np.asarray(hidden_states, dtype=np.float32)
    w = np.asarray(c_attn_w, dtype=np.float32)
    bvec = np.asarray(c_attn_b, dtype=np.float32)
    pw = np.asarray(c_proj_w, dtype=np.float32)
    pb = np.asarray(c_proj_b, dtype=np.float32)
    nc = _get_nc()
    res = run_bass_kernel_spmd(nc, make_in_maps(hs, w, bvec, pw),
                               core_ids=list(range(NCORES)), **run_kwargs)
    parts = [res.results[i]["out"] for i in range(NCORES)]
    out = combine(parts, bvec, pw, pb)
    if run_kwargs:
        return out, res
    return out
